# revision 11
# baseline (speedup 1.0000x reference)
"""GAT layer (B=8, N=2048, F=64) on 8 trn2 NeuronCores.

Strategy: data-parallel over batch B — one graph per core, adj replicated.

Math: with e = leaky_relu(e1_i + e2_j), exp(e - 0.2*e1_i) (row factor
cancels in softmax) = A2_j * max(G_i, r_j) where G = exp(0.8*e1),
A2 = exp(e2), r = exp(-0.8*e2). A2 folds into the matmul weights on the
host (whA = [Wh*A2 | A2]; row 64 yields softmax denominators), so the
device computes q_ij = max(G_i, r_j) * adj_ji and accumulates
outT[65, i] += whA_t^T @ q_t over 16 j-tiles. Divide + elu epilogue is
O(N*F) and runs on the host.

The binding resources are SBUF bandwidth (~7-9 B/ns/partition aggregate)
and DVE cycles, so the N^2 elementwise stage minimizes both:
  - 11 j-tiles: ONE fused DVE scalar_tensor_tensor each,
    q_t = (G max r_t) * adj_t (~2.35us, reads G fp16 + adj fp8, writes
    q fp16 — 10KB/partition/tile, no intermediate score).
  - 5 j-tiles (every 3rd): masked on the otherwise-idle Pool engine
    (q = s * adj, ~4.5us); their scores s = (G max r_t) run on DVE in
    4x mode (~0.65us) during the startup window while DVE would idle
    waiting for the first adj DMA anyway.
Startup/tail are trimmed: adj arrives as [1,3,4,4,4]-tile DMAs so tile 0
lands early; small inputs issue from the idle Scalar queue; the 16-tile
accumulation runs as two 4-bank phases (tiles 0-7, 8-15) whose PSUM
copies/output DMAs overlap the other phase; host adds the two halves.
G is partition-broadcast by DMA (stride-0 source AP). adj/q tiles are
SBUF-resident (no ring recycling -> minimal semaphore waits).
"""

import sys

import numpy as np
import ml_dtypes

for _p in ("/opt/trn_rl_repo",):
    if _p not in sys.path:
        sys.path.insert(0, _p)

from contextlib import ExitStack

import concourse.bass as bass
import concourse.tile as tile
from concourse import bacc, mybir
from concourse.bass_utils import run_bass_kernel_spmd

B, N, F = 8, 2048, 64
P = 128
T = N // P  # 16 j-tiles
NB = N // 512  # 4 psum banks of moving-free 512

POOL_TILES = (2, 5, 8, 11, 14)
ADJ_GROUPS = [(0,), (1, 2, 3), (4, 5, 6, 7), (8, 9, 10, 11), (12, 13, 14, 15)]

_CACHE = {}


def _build_program():
    if "nc" in _CACHE:
        return _CACHE["nc"]
    dt = mybir.dt
    nc = bacc.Bacc("TRN2", target_bir_lowering=False, debug=False)

    # adj^T tiles stacked along the free dim: tile t at columns [t*N, (t+1)*N).
    adjd = nc.dram_tensor("adjd", [P, T * N], dt.float8e4, kind="ExternalInput").ap()
    g = nc.dram_tensor("g", [1, N], dt.float16, kind="ExternalInput").ap()
    rsc = nc.dram_tensor("rsc", [P, T], dt.float32, kind="ExternalInput").ap()
    wha = nc.dram_tensor("wha", [P, T * 65], dt.float16, kind="ExternalInput").ap()
    outA = nc.dram_tensor("outA", [65, N], dt.float16, kind="ExternalOutput").ap()
    outB = nc.dram_tensor("outB", [65, N], dt.float16, kind="ExternalOutput").ap()

    with tile.TileContext(nc) as tc, ExitStack() as ctx:
        singles = ctx.enter_context(tc.tile_pool(name="singles", bufs=1))
        accp = ctx.enter_context(tc.tile_pool(name="accp", bufs=1, space="PSUM"))

        rsc_sb = singles.tile([P, T], dt.float32)
        nc.scalar.dma_start(out=rsc_sb[:], in_=rsc)
        g_sb = singles.tile([P, N], dt.float16)
        nc.scalar.dma_start(out=g_sb[:], in_=g.to_broadcast((P, N)))
        wha_sb = singles.tile([P, T * 65], dt.float16)
        nc.scalar.dma_start(out=wha_sb[:], in_=wha)

        adj_sb = singles.tile([P, T * N], dt.float8e4, name="adj")
        for grp in ADJ_GROUPS:
            lo, hi = grp[0], grp[-1] + 1
            nc.sync.dma_start(
                out=adj_sb[:, lo * N : hi * N], in_=adjd[:, lo * N : hi * N]
            )

        # Scores for Pool-masked tiles: DVE 4x mode, needs only g/rsc, so
        # they fill DVE's idle window before the first adj DMA lands.
        scores = {}
        for t in POOL_TILES:
            st = singles.tile([P, N], dt.float16, name=f"s{t}")
            nc.vector.tensor_scalar_max(st[:], g_sb[:], rsc_sb[:, t : t + 1])
            scores[t] = st

        accs = {}
        for ph in range(2):
            for n in range(NB):
                accs[ph, n] = accp.tile(
                    [65, 512], dt.float32, tag=f"acc{ph}_{n}", name=f"acc{ph}_{n}"
                )

        osbA = singles.tile([65, N], dt.float16, name="osbA")
        osbB = singles.tile([65, N], dt.float16, name="osbB")

        for t in range(T):
            ph, first, last = t // 8, t % 8 == 0, t % 8 == 7
            qt = singles.tile([P, N], dt.float16, name=f"q{t}")
            if t in POOL_TILES:
                nc.gpsimd.tensor_tensor(
                    qt[:], scores[t][:], adj_sb[:, t * N : (t + 1) * N],
                    mybir.AluOpType.mult,
                )
            else:
                nc.vector.scalar_tensor_tensor(
                    out=qt[:],
                    in0=g_sb[:],
                    scalar=rsc_sb[:, t : t + 1],
                    in1=adj_sb[:, t * N : (t + 1) * N],
                    op0=mybir.AluOpType.max,
                    op1=mybir.AluOpType.mult,
                )
            for n in range(NB):
                nc.tensor.matmul(
                    out=accs[ph, n][:],
                    lhsT=wha_sb[:, t * 65 : (t + 1) * 65],
                    rhs=qt[:, n * 512 : (n + 1) * 512],
                    start=first,
                    stop=last,
                )
            if last:
                osb, outd = (osbA, outA) if ph == 0 else (osbB, outB)
                for n in range(NB):
                    nc.scalar.copy(osb[:, n * 512 : (n + 1) * 512], accs[ph, n][:])
                nc.sync.dma_start(out=outd, in_=osb[:])

    nc.compile()
    _CACHE["nc"] = nc
    return nc


def _prep_inputs(h, adj, W, a):
    h = np.asarray(h, np.float32)
    adj = np.asarray(adj, np.float32)
    W = np.asarray(W, np.float32)
    a = np.asarray(a, np.float32)

    # adj^T tiles side by side along free dim: adjd[p, t*N + i] = adjT[t*128+p, i]
    adjd = np.ascontiguousarray(
        adj.T.reshape(T, P, N).transpose(1, 0, 2).reshape(P, T * N)
    ).astype(ml_dtypes.float8_e4m3)

    Wh = np.einsum("bnf,of->bno", h, W)  # [B, N, F]
    e1 = Wh @ a[:F]  # [B, N]
    e2 = Wh @ a[F:]  # [B, N]
    A2 = np.exp(e2)
    G = np.exp(0.8 * e1).astype(np.float16)  # [B, N]
    r = np.exp(-0.8 * e2).astype(np.float32)  # [B, N]
    whA = np.concatenate([Wh * A2[..., None], A2[..., None]], axis=2)  # [B, N, 65]
    whA = np.ascontiguousarray(
        whA.reshape(B, T, P, 65).transpose(0, 2, 1, 3)
    ).reshape(B, P, T * 65)

    in_maps = []
    for b in range(B):
        in_maps.append(
            {
                "adjd": adjd,
                "g": G[b].reshape(1, N),
                "rsc": np.ascontiguousarray(r[b].reshape(T, P).T),
                "wha": whA[b].astype(np.float16),
            }
        )
    return in_maps


def kernel(h, adj, W, a, _trace=False):
    nc = _build_program()
    in_maps = _prep_inputs(h, adj, W, a)
    res = run_bass_kernel_spmd(nc, in_maps, list(range(B)), trace=_trace)
    outs = np.empty((B, N, F), np.float32)
    for b in range(B):
        outT = np.asarray(res.results[b]["outA"], np.float32) + np.asarray(
            res.results[b]["outB"], np.float32
        )
        hp = outT[:F].T / outT[F][:, None]
        outs[b] = np.where(hp > 0, hp, np.expm1(hp))
    if _trace:
        kernel.last_results = res
    return outs


# revision 12
# speedup vs baseline: 1.2441x; 1.2441x over previous
"""GAT layer (B=8, N=2048, F=64) on 8 trn2 NeuronCores.

Strategy: data-parallel over batch B — one graph per core, adj replicated.

Math: with e = leaky_relu(e1_i + e2_j), exp(e - 0.2*e1_i) (row factor
cancels in softmax) = A2_j * max(G_i, r_j) where G = exp(0.8*e1),
A2 = exp(e2), r = exp(-0.8*e2). A2 folds into the matmul weights on the
host (whA = [Wh*A2 | A2]; row 64 yields softmax denominators), so the
device computes q_ij = max(G_i, r_j) * adj_ji and accumulates
outT[65, i] += whA_t^T @ q_t over 16 j-tiles. Divide + elu epilogue is
O(N*F) and runs on the host.

Engine mapping (measured on HW, not the cost model):
  - The whole N^2 stage runs on DVE: score s_t = (G max r_t) in 4x mode
    (~0.65us/tile) + mask q_t = s_t * adj_t in 2x mode (~1.2us/tile).
    Both fast modes need all-2-byte operands, hence adj in fp16.
    The fused scalar_tensor_tensor (1x, ~2.35us) and any GpSimd/Pool
    offload lose: Pool's software SBUF accesses wreck DVE's fast modes
    (measured 2.5-6x degradation when Pool runs concurrently).
  - Early scores are emitted first so DVE works while the first adj
    DMA is still in flight; small inputs issue from the idle Scalar
    queue; adj lands in [2,2,4,4,4]-tile chunks on the Sync queue.
  - The 16-tile PSUM accumulation runs as two 4-bank phases (tiles 0-7,
    8-15) whose ACT copies + output DMA overlap the other phase; the
    host adds the halves. G is partition-broadcast by DMA.
"""

import sys

import numpy as np

for _p in ("/opt/trn_rl_repo",):
    if _p not in sys.path:
        sys.path.insert(0, _p)

from contextlib import ExitStack

import concourse.bass as bass
import concourse.tile as tile
from concourse import bacc, mybir
from concourse.bass_utils import run_bass_kernel_spmd

B, N, F = 8, 2048, 64
P = 128
T = N // P  # 16 j-tiles
NB = N // 512  # 4 psum banks of moving-free 512

ADJ_GROUPS = [(0, 1), (2, 3), (4, 5, 6, 7), (8, 9, 10, 11), (12, 13, 14, 15)]
PREFETCH = 4  # scores emitted ahead of the mask loop

_CACHE = {}


def _build_program():
    if "nc" in _CACHE:
        return _CACHE["nc"]
    dt = mybir.dt
    nc = bacc.Bacc("TRN2", target_bir_lowering=False, debug=False)

    # adj^T tiles stacked along the free dim: tile t at columns [t*N, (t+1)*N).
    adjd = nc.dram_tensor("adjd", [P, T * N], dt.float16, kind="ExternalInput").ap()
    g = nc.dram_tensor("g", [1, N], dt.float16, kind="ExternalInput").ap()
    rsc = nc.dram_tensor("rsc", [P, T], dt.float32, kind="ExternalInput").ap()
    wha = nc.dram_tensor("wha", [P, T * 65], dt.float16, kind="ExternalInput").ap()
    outA = nc.dram_tensor("outA", [65, N], dt.float16, kind="ExternalOutput").ap()
    outB = nc.dram_tensor("outB", [65, N], dt.float16, kind="ExternalOutput").ap()

    with tile.TileContext(nc) as tc, ExitStack() as ctx:
        singles = ctx.enter_context(tc.tile_pool(name="singles", bufs=1))
        spool = ctx.enter_context(tc.tile_pool(name="spool", bufs=PREFETCH + 2))
        qpool = ctx.enter_context(tc.tile_pool(name="qpool", bufs=6))
        accp = ctx.enter_context(tc.tile_pool(name="accp", bufs=1, space="PSUM"))

        rsc_sb = singles.tile([P, T], dt.float32)
        nc.scalar.dma_start(out=rsc_sb[:], in_=rsc)
        g_sb = singles.tile([P, N], dt.float16)
        nc.scalar.dma_start(out=g_sb[:], in_=g.to_broadcast((P, N)))
        wha_sb = singles.tile([P, T * 65], dt.float16)
        nc.scalar.dma_start(out=wha_sb[:], in_=wha)

        adj_sb = singles.tile([P, T * N], dt.float16, name="adj")
        for grp in ADJ_GROUPS:
            lo, hi = grp[0], grp[-1] + 1
            nc.sync.dma_start(
                out=adj_sb[:, lo * N : hi * N], in_=adjd[:, lo * N : hi * N]
            )

        def make_score(t):
            st = spool.tile([P, N], dt.float16, name="s")
            nc.vector.tensor_scalar_max(st[:], g_sb[:], rsc_sb[:, t : t + 1])
            return st

        # Scores only need g/rsc: front-run them while adj DMAs land.
        scores = {t: make_score(t) for t in range(PREFETCH)}

        accs = {}
        for ph in range(2):
            for n in range(NB):
                accs[ph, n] = accp.tile(
                    [65, 512], dt.float32, tag=f"acc{ph}_{n}", name=f"acc{ph}_{n}"
                )

        osbA = singles.tile([65, N], dt.float16, name="osbA")
        osbB = singles.tile([65, N], dt.float16, name="osbB")

        for t in range(T):
            ph, first, last = t // 8, t % 8 == 0, t % 8 == 7
            st = scores.pop(t) if t in scores else make_score(t)
            qt = qpool.tile([P, N], dt.float16)
            nc.vector.tensor_tensor(
                qt[:], st[:], adj_sb[:, t * N : (t + 1) * N], mybir.AluOpType.mult
            )
            for n in range(NB):
                nc.tensor.matmul(
                    out=accs[ph, n][:],
                    lhsT=wha_sb[:, t * 65 : (t + 1) * 65],
                    rhs=qt[:, n * 512 : (n + 1) * 512],
                    start=first,
                    stop=last,
                )
            if last:
                osb, outd = (osbA, outA) if ph == 0 else (osbB, outB)
                for n in range(NB):
                    nc.scalar.copy(osb[:, n * 512 : (n + 1) * 512], accs[ph, n][:])
                nc.sync.dma_start(out=outd, in_=osb[:])

    nc.compile()
    _CACHE["nc"] = nc
    return nc


def _prep_inputs(h, adj, W, a):
    h = np.asarray(h, np.float32)
    adj = np.asarray(adj, np.float32)
    W = np.asarray(W, np.float32)
    a = np.asarray(a, np.float32)

    # adj^T tiles side by side along free dim: adjd[p, t*N + i] = adjT[t*128+p, i]
    adjd = np.ascontiguousarray(
        adj.T.reshape(T, P, N).transpose(1, 0, 2).reshape(P, T * N)
    ).astype(np.float16)

    Wh = np.einsum("bnf,of->bno", h, W)  # [B, N, F]
    e1 = Wh @ a[:F]  # [B, N]
    e2 = Wh @ a[F:]  # [B, N]
    A2 = np.exp(e2)
    G = np.exp(0.8 * e1).astype(np.float16)  # [B, N]
    r = np.exp(-0.8 * e2).astype(np.float32)  # [B, N]
    whA = np.concatenate([Wh * A2[..., None], A2[..., None]], axis=2)  # [B, N, 65]
    whA = np.ascontiguousarray(
        whA.reshape(B, T, P, 65).transpose(0, 2, 1, 3)
    ).reshape(B, P, T * 65)

    in_maps = []
    for b in range(B):
        in_maps.append(
            {
                "adjd": adjd,
                "g": G[b].reshape(1, N),
                "rsc": np.ascontiguousarray(r[b].reshape(T, P).T),
                "wha": whA[b].astype(np.float16),
            }
        )
    return in_maps


def kernel(h, adj, W, a, _trace=False):
    nc = _build_program()
    in_maps = _prep_inputs(h, adj, W, a)
    res = run_bass_kernel_spmd(nc, in_maps, list(range(B)), trace=_trace)
    outs = np.empty((B, N, F), np.float32)
    for b in range(B):
        outT = np.asarray(res.results[b]["outA"], np.float32) + np.asarray(
            res.results[b]["outB"], np.float32
        )
        hp = outT[:F].T / outT[F][:, None]
        outs[b] = np.where(hp > 0, hp, np.expm1(hp))
    if _trace:
        kernel.last_results = res
    return outs


# revision 19
# speedup vs baseline: 1.2499x; 1.0047x over previous
"""GAT layer (B=8, N=2048, F=64) on 8 trn2 NeuronCores.

Strategy: data-parallel over batch B — one graph per core, adj replicated.

Math: with e = leaky_relu(e1_i + e2_j), exp(e - 0.2*e1_i) (row factor
cancels in softmax) = A2_j * max(G_i, r_j) where G = exp(0.8*e1),
A2 = exp(e2), r = exp(-0.8*e2). A2 folds into the matmul weights on the
host (whA = [Wh*A2 | A2]; row 64 yields softmax denominators), so the
device computes q_ij = max(G_i, r_j) * adj_ji and accumulates
outT[65, i] += whA_t^T @ q_t over 16 j-tiles. Divide + elu epilogue is
O(N*F) and runs on the host.

Engine mapping (measured on HW, not the cost model):
  - The whole N^2 stage runs on DVE: score s_t = (G max r_t) in 4x mode
    (~0.65us/tile) + mask q_t = s_t * adj_t in 2x mode (~1.2us/tile).
    Both fast modes need all-2-byte operands, hence adj in fp16.
    The fused scalar_tensor_tensor (1x, ~2.35us) and any GpSimd/Pool
    offload lose: Pool's software SBUF accesses wreck DVE's fast modes
    (measured 2.5-6x degradation when Pool runs concurrently).
  - Early scores are emitted first so DVE works while the first adj
    DMA is still in flight; small inputs issue from the idle Scalar
    queue; adj lands in [2,2,4,4,4]-tile chunks on the Sync queue.
  - The 16-tile PSUM accumulation runs as two 4-bank phases (tiles 0-7,
    8-15) whose ACT copies + output DMA overlap the other phase; the
    host adds the halves. G is partition-broadcast by DMA.
"""

import sys

import numpy as np

for _p in ("/opt/trn_rl_repo",):
    if _p not in sys.path:
        sys.path.insert(0, _p)

from contextlib import ExitStack

import concourse.bass as bass
import concourse.tile as tile
from concourse import bacc, mybir
from concourse.bass_utils import run_bass_kernel_spmd

B, N, F = 8, 2048, 64
P = 128
T = N // P  # 16 j-tiles
NB = N // 512  # 4 psum banks of moving-free 512

ADJ_GROUPS = [tuple(range(k, k + 2)) for k in range(0, T, 2)]
PREFETCH = 4  # scores emitted ahead of the mask loop

_CACHE = {}


def _build_program():
    if "nc" in _CACHE:
        return _CACHE["nc"]
    dt = mybir.dt
    nc = bacc.Bacc("TRN2", target_bir_lowering=False, debug=False)

    # adj^T tiles stacked along the free dim: tile t at columns [t*N, (t+1)*N).
    adjd = nc.dram_tensor("adjd", [P, T * N], dt.float16, kind="ExternalInput").ap()
    g = nc.dram_tensor("g", [P, N], dt.float16, kind="ExternalInput").ap()
    rsc = nc.dram_tensor("rsc", [P, T], dt.float32, kind="ExternalInput").ap()
    wha = nc.dram_tensor("wha", [P, T * 65], dt.float16, kind="ExternalInput").ap()
    outA = nc.dram_tensor("outA", [65, N], dt.float16, kind="ExternalOutput").ap()
    outB = nc.dram_tensor("outB", [65, N], dt.float16, kind="ExternalOutput").ap()

    with tile.TileContext(nc) as tc, ExitStack() as ctx:
        singles = ctx.enter_context(tc.tile_pool(name="singles", bufs=1))
        spool = ctx.enter_context(tc.tile_pool(name="spool", bufs=PREFETCH + 4))
        qpool = ctx.enter_context(tc.tile_pool(name="qpool", bufs=6))
        accp = ctx.enter_context(tc.tile_pool(name="accp", bufs=1, space="PSUM"))

        rsc_sb = singles.tile([P, T], dt.float32)
        nc.scalar.dma_start(out=rsc_sb[:], in_=rsc)
        g_sb = singles.tile([P, N], dt.float16)
        nc.scalar.dma_start(out=g_sb[:], in_=g)
        wha_sb = singles.tile([P, T * 65], dt.float16)
        nc.scalar.dma_start(out=wha_sb[:], in_=wha)

        adj_sb = singles.tile([P, T * N], dt.float16, name="adj")
        for grp in ADJ_GROUPS:
            lo, hi = grp[0], grp[-1] + 1
            nc.sync.dma_start(
                out=adj_sb[:, lo * N : hi * N], in_=adjd[:, lo * N : hi * N]
            )

        def make_score(t):
            st = spool.tile([P, N], dt.float16, name="s")
            nc.vector.tensor_scalar_max(st[:], g_sb[:], rsc_sb[:, t : t + 1])
            return st

        # Scores only need g/rsc: front-run them while adj DMAs land.
        scores = {t: make_score(t) for t in range(PREFETCH)}

        accs = {}
        for ph in range(2):
            for n in range(NB):
                accs[ph, n] = accp.tile(
                    [65, 512], dt.float32, tag=f"acc{ph}_{n}", name=f"acc{ph}_{n}"
                )

        osbA = singles.tile([65, N], dt.float16, name="osbA")
        osbB = singles.tile([65, N], dt.float16, name="osbB")

        for t in range(T):
            ph, first, last = t // 8, t % 8 == 0, t % 8 == 7
            st = scores.pop(t) if t in scores else make_score(t)
            qt = qpool.tile([P, N], dt.float16)
            nc.vector.tensor_tensor(
                qt[:], st[:], adj_sb[:, t * N : (t + 1) * N], mybir.AluOpType.mult
            )
            for n in range(NB):
                nc.tensor.matmul(
                    out=accs[ph, n][:],
                    lhsT=wha_sb[:, t * 65 : (t + 1) * 65],
                    rhs=qt[:, n * 512 : (n + 1) * 512],
                    start=first,
                    stop=last,
                )
            if last:
                osb, outd = (osbA, outA) if ph == 0 else (osbB, outB)
                for n in range(NB):
                    # Split PSUM->SBUF copies over ACT and DVE (DVE is done
                    # with its stream by the time the last bank stops).
                    dst = osb[:, n * 512 : (n + 1) * 512]
                    if n % 2 == 0:
                        nc.scalar.copy(dst, accs[ph, n][:])
                    else:
                        nc.vector.tensor_copy(dst, accs[ph, n][:])
                nc.sync.dma_start(out=outd, in_=osb[:])

    nc.compile()
    _CACHE["nc"] = nc
    return nc


def _prep_inputs(h, adj, W, a):
    h = np.asarray(h, np.float32)
    adj = np.asarray(adj, np.float32)
    W = np.asarray(W, np.float32)
    a = np.asarray(a, np.float32)

    # adj^T tiles side by side along free dim: adjd[p, t*N + i] = adjT[t*128+p, i]
    adjd = np.ascontiguousarray(
        adj.T.reshape(T, P, N).transpose(1, 0, 2).reshape(P, T * N)
    ).astype(np.float16)

    Wh = np.einsum("bnf,of->bno", h, W)  # [B, N, F]
    e1 = Wh @ a[:F]  # [B, N]
    e2 = Wh @ a[F:]  # [B, N]
    A2 = np.exp(e2)
    G = np.exp(0.8 * e1).astype(np.float16)  # [B, N]
    r = np.exp(-0.8 * e2).astype(np.float32)  # [B, N]
    whA = np.concatenate([Wh * A2[..., None], A2[..., None]], axis=2)  # [B, N, 65]
    whA = np.ascontiguousarray(
        whA.reshape(B, T, P, 65).transpose(0, 2, 1, 3)
    ).reshape(B, P, T * 65)

    in_maps = []
    for b in range(B):
        in_maps.append(
            {
                "adjd": adjd,
                "g": np.ascontiguousarray(np.broadcast_to(G[b], (P, N))),
                "rsc": np.ascontiguousarray(r[b].reshape(T, P).T),
                "wha": whA[b].astype(np.float16),
            }
        )
    return in_maps


def kernel(h, adj, W, a, _trace=False):
    nc = _build_program()
    in_maps = _prep_inputs(h, adj, W, a)
    res = run_bass_kernel_spmd(nc, in_maps, list(range(B)), trace=_trace)
    outs = np.empty((B, N, F), np.float32)
    for b in range(B):
        outT = np.asarray(res.results[b]["outA"], np.float32) + np.asarray(
            res.results[b]["outB"], np.float32
        )
        hp = outT[:F].T / outT[F][:, None]
        outs[b] = np.where(hp > 0, hp, np.expm1(hp))
    if _trace:
        kernel.last_results = res
    return outs
